# revision 10
# baseline (speedup 1.0000x reference)
"""Causal self-attention on 8 TRN2 NeuronCores.

Sharding: core c = (batch b = c//2, head-group g = c%2).  Each core computes
the full attention for one batch and 8 of the 16 heads (column-sharded
Wq/Wk/Wv, row-sharded Wproj), producing a partial output projection; the two
partials per batch are summed on the host (the row-parallel all-reduce).

Matmul operands are bf16 (fp32 psum accumulation).  Per-core dataflow:
  xT[c_in, t]  (host pre-transposed, bf16, split into two 512-col halves)
  qT/kT[cq, t] = Wq/Wk^T @ xT          (pair-packed: 2 heads per 128-part tile)
  v[t, cv]     = x @ Wv                (stored [t, head, 65] with ones column)
  scoresT[k,q] = k @ qT  per head      (row-group-packed pair matmuls, K=64,
                                        both heads of a pair issued
                                        back-to-back so they overlap on the PE)
  expT         = exp(0.125*scoresT) into a [128, 2, 512] pair tile; diagonal
                 blocks masked post-exp by one 0/1 lower-triangle multiply on
                 GpSimd covering both heads
  outT[dv,q],sums[q] = [v|1].T @ expT  (psum accumulate over k tiles)
  normalize    = fp32 reciprocal of the sums row straight from PSUM, K=1
                 outer-product broadcast matmuls, DVE multiplies
  y_partial    = outT_scaled.T @ Wproj_rows   (bf16 output, host sums in f32)

The attention inner loop is paced by ScalarE (exp); remaining QKV / output
projection matmul groups are generators "pumped" between attention steps so
the TensorE stream stays dense (and the HAM clock stays up).  All input DMAs
ride one queue in first-use order (the 16 SDMA rings serialize anyway); a few
warm-up matmuls on the constant tiles burn the cold-clock window while the
first real operands stream in.
"""

import numpy as np
import ml_dtypes
from contextlib import ExitStack

import concourse.tile as tile
from concourse import bacc, mybir
from concourse.bass import ts
from concourse.bass_utils import run_bass_kernel_spmd

F32 = mybir.dt.float32
BF16 = mybir.dt.bfloat16
AF = mybir.ActivationFunctionType

N_CORES = 8
T = 1024
C = 1024
D = 64          # head dim
HL = 8          # heads per core
CL = HL * D     # 512 local channels
NKT = 8         # k (key) tiles of 128
NPAIR = 4       # head pairs per core

_CACHE = {}


def _build():
    nc = bacc.Bacc("TRN2", target_bir_lowering=False, debug=False,
                   num_devices=N_CORES)
    xt = nc.dram_tensor("xt", [C, T], BF16, kind="ExternalInput").ap()
    wq = nc.dram_tensor("wq", [C, CL], BF16, kind="ExternalInput").ap()
    wk = nc.dram_tensor("wk", [C, CL], BF16, kind="ExternalInput").ap()
    wv = nc.dram_tensor("wv", [C, CL], BF16, kind="ExternalInput").ap()
    wp = nc.dram_tensor("wp", [CL, C], BF16, kind="ExternalInput").ap()
    # 0/1 keep-mask: tri[p, f] = 1 where f >= p (two copies side by side)
    tri = nc.dram_tensor("tri", [128, 256], BF16, kind="ExternalInput").ap()
    ones_a = nc.dram_tensor("ones_a", [1, 64], F32, kind="ExternalInput").ap()
    ones_c = nc.dram_tensor("ones_c", [1, 64], BF16, kind="ExternalInput").ap()
    ones_b = nc.dram_tensor("ones_b", [128, 8], BF16, kind="ExternalInput").ap()
    y = nc.dram_tensor("y", [T, C], BF16, kind="ExternalOutput").ap()

    xt_r = xt.rearrange("(kt p) t -> kt p t", p=128)
    w_r = {n: w.rearrange("(kt p) n -> kt p n", p=128)
           for n, w in (("wq", wq), ("wk", wk), ("wv", wv))}

    with tile.TileContext(nc) as tc, ExitStack() as ctx:
        const = ctx.enter_context(tc.tile_pool(name="const", bufs=1))
        big = ctx.enter_context(tc.tile_pool(name="big", bufs=1))
        ps_main = ctx.enter_context(
            tc.tile_pool(name="ps_main", bufs=2, space="PSUM"))
        ps_out = ctx.enter_context(
            tc.tile_pool(name="ps_out", bufs=2, space="PSUM"))
        sb_exp = ctx.enter_context(tc.tile_pool(name="sb_exp", bufs=6))
        sb_tmp = ctx.enter_context(tc.tile_pool(name="sb_tmp", bufs=4))
        sb_y = ctx.enter_context(tc.tile_pool(name="sb_y", bufs=4))

        # ---- load phase ----
        # Single queue, first-use order: the 16 SDMA rings serialize all
        # transfers anyway; what matters is that the first-needed tiles are
        # first on the ring.
        ones_b_sb = const.tile([128, 8], BF16)
        nc.sync.dma_start(out=ones_b_sb[:], in_=ones_b)
        ones_a_sb = const.tile([65, 64], F32)
        nc.sync.dma_start(out=ones_a_sb[64:65, :], in_=ones_a)
        ones_c_sb = const.tile([65, 64], BF16)
        nc.sync.dma_start(out=ones_c_sb[64:65, :], in_=ones_c)
        tri_sb = const.tile([128, 2, 128], BF16)
        nc.sync.dma_start(
            out=tri_sb[:], in_=tri.rearrange("p (b f) -> p b f", b=2))

        wv_sb, wq_sb, wk_sb = [], [], []
        xa_sb, xb_sb = [], []
        for kt in range(NKT):
            wc = big.tile([128, CL], BF16, name=f"wv{kt}")
            nc.sync.dma_start(out=wc[:], in_=w_r["wv"][kt])
            wv_sb.append(wc)
            xc = big.tile([128, 512], BF16, name=f"xa{kt}")
            nc.sync.dma_start(out=xc[:], in_=xt_r[kt][:, 0:512])
            xa_sb.append(xc)
        for kt in range(NKT):
            wc = big.tile([128, CL], BF16, name=f"wq{kt}")
            nc.sync.dma_start(out=wc[:], in_=w_r["wq"][kt])
            wq_sb.append(wc)
        for kt in range(NKT):
            wc = big.tile([128, CL], BF16, name=f"wk{kt}")
            nc.sync.dma_start(out=wc[:], in_=w_r["wk"][kt])
            wk_sb.append(wc)
        for kt in range(NKT):
            xc = big.tile([128, 512], BF16, name=f"xb{kt}")
            nc.sync.dma_start(out=xc[:], in_=xt_r[kt][:, 512:1024])
            xb_sb.append(xc)
        wp_sb = big.tile([128, NPAIR, C], BF16)
        nc.sync.dma_start(
            out=wp_sb[:], in_=wp.rearrange("(r p) n -> p r n", p=128))

        x_half = {0: xa_sb, 1: xb_sb}

        qT_sb = big.tile([128, NPAIR, T], BF16)
        kT_sb = big.tile([128, NPAIR, T], BF16)
        v_sb = big.tile([128, NKT, HL, D + 1], BF16)
        projT_sb = big.tile([128, NPAIR, T], BF16)

        # ---- PE warm-up: a memset tile needs no DMA, so these matmuls start
        # right away, keep the PE busy until the first operands land, and get
        # the HAM clock to 2.4 GHz before the real work begins.
        wu_sb = const.tile([128, 128], BF16)
        nc.vector.memset(wu_sb[:], 0)
        warm_ps = ps_main.tile([128, 512], F32, name="warm", tag="fl")
        for _ in range(36):
            nc.tensor.matmul(warm_ps[0:8, 0:128], wu_sb[:, 0:8],
                             wu_sb[:, :], start=True, stop=True)

        # ---- PE work generators ----
        def qkv_group(dst, w_sb, m, nt):
            ps = ps_main.tile([128, 512], F32, name="ps", tag="fl")
            for kt in range(NKT):
                nc.tensor.matmul(
                    ps[:], w_sb[kt][:, ts(m, 128)],
                    x_half[nt][kt][:],
                    start=(kt == 0), stop=(kt == NKT - 1))
                if kt % 2 == 1:
                    yield
            nc.vector.tensor_copy(dst[:, m, ts(nt, 512)], ps[:])

        def v_group(tt):
            ps = ps_main.tile([128, 512], F32, name="ps", tag="fl")
            half, off = (xa_sb, 0) if tt < 4 else (xb_sb, 512)
            for kt in range(NKT):
                nc.tensor.matmul(
                    ps[:], half[kt][:, 128 * tt - off:128 * (tt + 1) - off],
                    wv_sb[kt][:],
                    start=(kt == 0), stop=(kt == NKT - 1))
                if kt % 2 == 1:
                    yield
            nc.vector.tensor_copy(
                v_sb[:, tt, :, 0:D],
                ps[:].rearrange("p (h d) -> p h d", h=HL))
            nc.vector.tensor_copy(v_sb[:, tt, :, D], ones_b_sb[:])

        def proj_group(q0, tt2, n2, drain=False):
            ps = ps_main.tile([128, 512], F32, name="ps", tag="fl")
            for r in range(NPAIR):
                nc.tensor.matmul(
                    ps[:],
                    projT_sb[:, r, q0 + 128 * tt2:q0 + 128 * (tt2 + 1)],
                    wp_sb[:, r, ts(n2, 512)],
                    start=(r == 0), stop=(r == NPAIR - 1))
                if r % 2 == 1:
                    yield
            yt = sb_y.tile([128, 512], BF16)
            if drain:
                nc.scalar.copy(yt[:], ps[:])
            else:
                nc.vector.tensor_copy(yt[:], ps[:])
            nc.sync.dma_start(
                out=y[q0 + 128 * tt2:q0 + 128 * (tt2 + 1), ts(n2, 512)],
                in_=yt[:])

        fillers = []  # [tag, generator]

        def pump(n):
            while n > 0 and fillers:
                tag, g = fillers[0]
                try:
                    next(g)
                    n -= 1
                except StopIteration:
                    fillers.pop(0)

        def flush(tags):
            i = 0
            while i < len(fillers):
                tag, g = fillers[i]
                if tag in tags:
                    for _ in g:
                        pass
                    fillers.pop(i)
                else:
                    i += 1

        # ---- QKV upfront: v/q/k for the first attention work; rest queued
        # as PE filler.  qt=0 only needs the nt=0 halves, so it can start
        # while the nt=1 operands are still streaming in.
        for tt in range(4):
            for _ in v_group(tt):
                pass
        for _ in qkv_group(qT_sb, wq_sb, 0, 0):
            pass
        for _ in qkv_group(kT_sb, wk_sb, 0, 0):
            pass
        for tt in range(4, NKT):
            for _ in v_group(tt):
                pass
        fillers.append(("q0n1", qkv_group(qT_sb, wq_sb, 0, 1)))
        fillers.append(("k0n1", qkv_group(kT_sb, wk_sb, 0, 1)))
        for m in range(1, NPAIR):
            fillers.append((f"k{m}n0", qkv_group(kT_sb, wk_sb, m, 0)))
            fillers.append((f"q{m}n0", qkv_group(qT_sb, wq_sb, m, 0)))
        for m in range(1, NPAIR):
            fillers.append((f"k{m}n1", qkv_group(kT_sb, wk_sb, m, 1)))
            fillers.append((f"q{m}n1", qkv_group(qT_sb, wq_sb, m, 1)))

        # ---- attention ----
        import os
        _V = os.environ.get("KV", "sbufmul")
        TAIL_SBUF = "sbufmul" in _V
        MASK2D = "mask2d" in _V

        def make_tail(m, outAB, q0):
            def tail():
                if TAIL_SBUF:
                    # sums rows -> bf16 (one op for both heads), K=1 broadcast
                    # matmuls, reciprocal into SBUF, multiply PSUM x SBUF
                    rr = sb_tmp.tile([65, 2, 512], BF16, name="rr")
                    nc.vector.tensor_copy(rr[64:65, :, :], outAB[64:65, :, :])
                    pump(2)
                    bcrs = []
                    for hh in range(2):
                        bc = ps_main.tile([64, 512], F32, name="bc", tag="fl")
                        nc.tensor.matmul(
                            bc[:], ones_c_sb[64:65, :], rr[64:65, hh, :],
                            start=True, stop=True, tile_position=(64, 0))
                        bcr = sb_tmp.tile([64, 512], F32, name="bcr")
                        nc.vector.reciprocal_approx_fast(out=bcr[:], in_=bc[:])
                        bcrs.append(bcr)
                    pump(1)
                    nc.vector.tensor_mul(
                        projT_sb[0:64, m, q0:q0 + 512],
                        outAB[0:64, 0, :], bcrs[0][:])
                    t2 = sb_tmp.tile([64, 512], BF16, name="t2")
                    nc.vector.tensor_mul(t2[:], outAB[0:64, 1, :], bcrs[1][:])
                else:
                    rr = sb_tmp.tile([65, 2, 512], F32, name="rr")
                    nc.vector.reciprocal_approx_fast(
                        out=rr[64:65, :, :], in_=outAB[64:65, :, :])
                    pump(2)
                    bcs = []
                    for hh in range(2):
                        bc = ps_main.tile([64, 512], F32, name="bc", tag="fl")
                        nc.tensor.matmul(
                            bc[:], ones_a_sb[64:65, :], rr[64:65, hh, :],
                            start=True, stop=True, tile_position=(64, 0))
                        bcs.append(bc)
                    pump(1)
                    nc.vector.tensor_mul(
                        projT_sb[0:64, m, q0:q0 + 512],
                        outAB[0:64, 0, :], bcs[0][:])
                    t2 = sb_tmp.tile([64, 512], BF16, name="t2")
                    nc.vector.tensor_mul(t2[:], outAB[0:64, 1, :], bcs[1][:])
                nc.sync.dma_start(
                    out=projT_sb[64:128, m, q0:q0 + 512], in_=t2[:])
            return tail

        for qt in (0, 1):
            q0 = 512 * qt
            pend_tail = None
            for m in range(NPAIR):
                flush({f"q{m}n{qt}", f"k{m}n0", f"k{m}n{qt}"})
                kts = list(range(4 * qt + 4))
                outAB = ps_out.tile([65, 2, 512], F32)
                pend = []
                for i in list(range(len(kts))) + [None]:
                    if i is not None:
                        kt = kts[i]
                        off = max(0, 128 * kt - q0)
                        w = 512 - off
                        qcols = slice(q0 + off, q0 + 512)
                        # score pair back-to-back so the row-tiled matmuls
                        # overlap on the PE
                        sAB = []
                        for hh, po in ((0, 0), (1, 64)):
                            s = ps_main.tile(
                                [128, 512], F32, name="sc", tag="sc")[:, :w]
                            nc.tensor.matmul(
                                s,
                                kT_sb[po:po + 64, m, ts(kt, 128)],
                                qT_sb[po:po + 64, m, qcols],
                                start=True, stop=True,
                                tile_position=(po, 0))
                            sAB.append(s)
                        eAB = sb_exp.tile([128, 2, 512], BF16, name="et")
                        for hh in range(2):
                            nc.scalar.activation(
                                eAB[:, hh, :w], sAB[hh], AF.Exp, scale=0.125)
                        if kt >= 4 * qt:  # diagonal: zero upper triangle
                            if MASK2D:
                                for hh in range(2):
                                    nc.gpsimd.tensor_mul(
                                        eAB[:, hh, 0:128], eAB[:, hh, 0:128],
                                        tri_sb[:, 0, :])
                            else:
                                nc.gpsimd.tensor_mul(
                                    eAB[:, :, 0:128], eAB[:, :, 0:128],
                                    tri_sb[:, :, :])
                        pend.append((eAB, i, off))
                    pump(1)
                    if len(pend) > 1 or (i is None and pend):
                        (eAB_p, pi, poff) = pend.pop(0)
                        pw = 512 - poff
                        for hh in range(2):
                            nc.tensor.matmul(
                                outAB[0:65, hh, poff:512],
                                v_sb[:, kts[pi], 2 * m + hh, :],
                                eAB_p[:, hh, 0:pw],
                                start=(pi == 0), stop=(pi == len(kts) - 1))
                    # previous pair's normalize, interleaved into this loop
                    if pend_tail is not None and (i == 1 or i is None):
                        pend_tail()
                        pend_tail = None
                pend_tail = make_tail(m, outAB, q0)
            pend_tail()
            pend_tail = None
            # queue this q-half's projection as PE filler for the next phase
            for tt2 in range(4):
                for n2 in range(2):
                    fillers.append((f"p{qt}", proj_group(
                        q0, tt2, n2, drain=(qt == 1))))
        # drain remaining projection work
        while fillers:
            pump(len(fillers) * 8)

    nc.compile()
    return nc


def _program():
    if "nc" not in _CACHE:
        _CACHE["nc"] = _build()
    return _CACHE["nc"]


def _bf(a):
    return np.ascontiguousarray(a).astype(ml_dtypes.bfloat16)


def _in_maps(x, Wq, Wk, Wv, Wproj):
    tri1 = np.triu(np.ones((128, 128), dtype=np.float32))  # tri[p,f]=1, f>=p
    tri = np.ascontiguousarray(np.concatenate([tri1, tri1], axis=1)
                               ).astype(ml_dtypes.bfloat16)
    ones_a = np.ones((1, 64), dtype=np.float32)
    ones_c = np.ones((1, 64), dtype=ml_dtypes.bfloat16)
    ones_b = np.ones((128, 8), dtype=ml_dtypes.bfloat16)
    maps = []
    for c in range(N_CORES):
        b, g = c // 2, c % 2
        sl = slice(CL * g, CL * (g + 1))
        maps.append({
            "xt": _bf(x[b].T),
            "wq": _bf(Wq[:, sl]),
            "wk": _bf(Wk[:, sl]),
            "wv": _bf(Wv[:, sl]),
            "wp": _bf(Wproj[sl, :]),
            "tri": tri,
            "ones_a": ones_a,
            "ones_c": ones_c,
            "ones_b": ones_b,
        })
    return maps


def run(x, Wq, Wk, Wv, Wproj, trace=False, **kwargs):
    nc = _program()
    maps = _in_maps(np.asarray(x, dtype=np.float32),
                    np.asarray(Wq, dtype=np.float32),
                    np.asarray(Wk, dtype=np.float32),
                    np.asarray(Wv, dtype=np.float32),
                    np.asarray(Wproj, dtype=np.float32))
    res = run_bass_kernel_spmd(nc, maps, core_ids=list(range(N_CORES)),
                               trace=trace, **kwargs)
    B = 4
    out = np.empty((B, T, C), dtype=np.float32)
    for b in range(B):
        out[b] = (res.results[2 * b]["y"].astype(np.float32)
                  + res.results[2 * b + 1]["y"].astype(np.float32))
    return out, res


def kernel(x, Wq, Wk, Wv, Wproj):
    out, _ = run(x, Wq, Wk, Wv, Wproj)
    return out


# revision 15
# speedup vs baseline: 1.0594x; 1.0594x over previous
"""Causal self-attention on 8 TRN2 NeuronCores.

Sharding: core c = (batch b = c//2, head-group g = c%2).  Each core computes
the full attention for one batch and 8 of the 16 heads (column-sharded
Wq/Wk/Wv, row-sharded Wproj), producing a partial output projection; the two
partials per batch are summed on the host (the row-parallel all-reduce).

Matmul operands are bf16 (fp32 psum accumulation).  Per-core dataflow:
  xT[c_in, t]  (host pre-transposed, bf16, split into two 512-col halves)
  qT/kT[cq, t] = Wq/Wk^T @ xT          (pair-packed: 2 heads per 128-part tile)
  v[t, cv]     = x @ Wv                (stored [t, head, 65] with ones column)
  scoresT[k,q] = k @ qT  per head      (row-group-packed pair matmuls, K=64,
                                        both heads of a pair issued
                                        back-to-back so they overlap on the PE)
  expT         = exp(0.125*scoresT) into a [128, 2, 512] pair tile; diagonal
                 blocks masked post-exp by one 0/1 lower-triangle multiply on
                 GpSimd covering both heads
  outT[dv,q],sums[q] = [v|1].T @ expT  (psum accumulate over k tiles)
  normalize    = fp32 reciprocal of the sums row straight from PSUM, K=1
                 outer-product broadcast matmuls, DVE multiplies
  y_partial    = outT_scaled.T @ Wproj_rows   (bf16 output, host sums in f32)

The attention inner loop is paced by ScalarE (exp); remaining QKV / output
projection matmul groups are generators "pumped" between attention steps so
the TensorE stream stays dense (and the HAM clock stays up).  All input DMAs
ride one queue in first-use order (the 16 SDMA rings serialize anyway); a few
warm-up matmuls on the constant tiles burn the cold-clock window while the
first real operands stream in.
"""

import numpy as np
import ml_dtypes
from contextlib import ExitStack

import concourse.tile as tile
from concourse import bacc, mybir
from concourse.bass import ts
from concourse.bass_utils import run_bass_kernel_spmd

F32 = mybir.dt.float32
BF16 = mybir.dt.bfloat16
AF = mybir.ActivationFunctionType

N_CORES = 8
T = 1024
C = 1024
D = 64          # head dim
HL = 8          # heads per core
CL = HL * D     # 512 local channels
NKT = 8         # k (key) tiles of 128
NPAIR = 4       # head pairs per core

_CACHE = {}


def _build():
    nc = bacc.Bacc("TRN2", target_bir_lowering=False, debug=False,
                   num_devices=N_CORES)
    xt = nc.dram_tensor("xt", [C, T], BF16, kind="ExternalInput").ap()
    wq = nc.dram_tensor("wq", [C, CL], BF16, kind="ExternalInput").ap()
    wk = nc.dram_tensor("wk", [C, CL], BF16, kind="ExternalInput").ap()
    wv = nc.dram_tensor("wv", [C, CL], BF16, kind="ExternalInput").ap()
    wp = nc.dram_tensor("wp", [CL, C], BF16, kind="ExternalInput").ap()
    # 0/1 keep-mask: tri[p, f] = 1 where f >= p (two copies side by side)
    tri = nc.dram_tensor("tri", [128, 256], BF16, kind="ExternalInput").ap()
    ones_a = nc.dram_tensor("ones_a", [1, 64], F32, kind="ExternalInput").ap()
    ones_c = nc.dram_tensor("ones_c", [1, 64], BF16, kind="ExternalInput").ap()
    ones_b = nc.dram_tensor("ones_b", [128, 8], BF16, kind="ExternalInput").ap()
    y = nc.dram_tensor("y", [T, C], BF16, kind="ExternalOutput").ap()

    xt_r = xt.rearrange("(kt p) t -> kt p t", p=128)
    w_r = {n: w.rearrange("(kt p) n -> kt p n", p=128)
           for n, w in (("wq", wq), ("wk", wk), ("wv", wv))}

    with tile.TileContext(nc) as tc, ExitStack() as ctx:
        const = ctx.enter_context(tc.tile_pool(name="const", bufs=1))
        big = ctx.enter_context(tc.tile_pool(name="big", bufs=1))
        ps_main = ctx.enter_context(
            tc.tile_pool(name="ps_main", bufs=4, space="PSUM"))
        ps_out = ctx.enter_context(
            tc.tile_pool(name="ps_out", bufs=2, space="PSUM"))
        sb_exp = ctx.enter_context(tc.tile_pool(name="sb_exp", bufs=6))
        sb_tmp = ctx.enter_context(tc.tile_pool(name="sb_tmp", bufs=4))
        sb_y = ctx.enter_context(tc.tile_pool(name="sb_y", bufs=4))

        # ---- load phase ----
        # Single queue, first-use order: the 16 SDMA rings serialize all
        # transfers anyway; what matters is that the first-needed tiles are
        # first on the ring.
        ones_b_sb = const.tile([128, 8], BF16)
        nc.sync.dma_start(out=ones_b_sb[:], in_=ones_b)
        ones_a_sb = const.tile([65, 64], F32)
        nc.sync.dma_start(out=ones_a_sb[64:65, :], in_=ones_a)
        ones_c_sb = const.tile([65, 64], BF16)
        nc.sync.dma_start(out=ones_c_sb[64:65, :], in_=ones_c)
        tri_sb = const.tile([128, 2, 128], BF16)
        nc.sync.dma_start(
            out=tri_sb[:], in_=tri.rearrange("p (b f) -> p b f", b=2))

        wv_sb, wq_sb, wk_sb = [], [], []
        xa_sb, xb_sb = [], []
        for kt in range(NKT):
            wc = big.tile([128, CL], BF16, name=f"wv{kt}")
            nc.sync.dma_start(out=wc[:], in_=w_r["wv"][kt])
            wv_sb.append(wc)
            xc = big.tile([128, 512], BF16, name=f"xa{kt}")
            nc.sync.dma_start(out=xc[:], in_=xt_r[kt][:, 0:512])
            xa_sb.append(xc)
        for kt in range(NKT):
            wc = big.tile([128, CL], BF16, name=f"wq{kt}")
            nc.sync.dma_start(out=wc[:], in_=w_r["wq"][kt])
            wq_sb.append(wc)
        for kt in range(NKT):
            wc = big.tile([128, CL], BF16, name=f"wk{kt}")
            nc.sync.dma_start(out=wc[:], in_=w_r["wk"][kt])
            wk_sb.append(wc)
        for kt in range(NKT):
            xc = big.tile([128, 512], BF16, name=f"xb{kt}")
            nc.sync.dma_start(out=xc[:], in_=xt_r[kt][:, 512:1024])
            xb_sb.append(xc)
        wp_sb = big.tile([128, NPAIR, C], BF16)
        nc.sync.dma_start(
            out=wp_sb[:], in_=wp.rearrange("(r p) n -> p r n", p=128))

        x_half = {0: xa_sb, 1: xb_sb}

        qT_sb = big.tile([128, NPAIR, T], BF16)
        kT_sb = big.tile([128, NPAIR, T], BF16)
        v_sb = big.tile([128, NKT, HL, D + 1], BF16)
        projT_sb = big.tile([128, NPAIR, T], BF16)

        # ---- PE work generators ----
        def qkv_group(dst, w_sb, m, nt):
            ps = ps_main.tile([128, 512], F32, name="ps")
            for kt in range(NKT):
                nc.tensor.matmul(
                    ps[:], w_sb[kt][:, ts(m, 128)],
                    x_half[nt][kt][:],
                    start=(kt == 0), stop=(kt == NKT - 1))
                if kt % 2 == 1:
                    yield
            nc.vector.tensor_copy(dst[:, m, ts(nt, 512)], ps[:])

        def v_group(tt):
            ps = ps_main.tile([128, 512], F32, name="ps")
            half, off = (xa_sb, 0) if tt < 4 else (xb_sb, 512)
            for kt in range(NKT):
                nc.tensor.matmul(
                    ps[:], half[kt][:, 128 * tt - off:128 * (tt + 1) - off],
                    wv_sb[kt][:],
                    start=(kt == 0), stop=(kt == NKT - 1))
                if kt % 2 == 1:
                    yield
            nc.vector.tensor_copy(
                v_sb[:, tt, :, 0:D],
                ps[:].rearrange("p (h d) -> p h d", h=HL))
            nc.vector.tensor_copy(v_sb[:, tt, :, D], ones_b_sb[:])

        def proj_group(q0, tt2, n2, drain=False):
            ps = ps_main.tile([128, 512], F32, name="ps")
            for r in range(NPAIR):
                nc.tensor.matmul(
                    ps[:],
                    projT_sb[:, r, q0 + 128 * tt2:q0 + 128 * (tt2 + 1)],
                    wp_sb[:, r, ts(n2, 512)],
                    start=(r == 0), stop=(r == NPAIR - 1))
                if r % 2 == 1:
                    yield
            yt = sb_y.tile([128, 512], BF16)
            if drain:
                nc.scalar.copy(yt[:], ps[:])
            else:
                nc.vector.tensor_copy(yt[:], ps[:])
            nc.sync.dma_start(
                out=y[q0 + 128 * tt2:q0 + 128 * (tt2 + 1), ts(n2, 512)],
                in_=yt[:])

        fillers = []  # [tag, generator]

        def pump(n):
            while n > 0 and fillers:
                tag, g = fillers[0]
                try:
                    next(g)
                    n -= 1
                except StopIteration:
                    fillers.pop(0)

        def flush(tags):
            i = 0
            while i < len(fillers):
                tag, g = fillers[i]
                if tag in tags:
                    for _ in g:
                        pass
                    fillers.pop(i)
                else:
                    i += 1

        # ---- QKV upfront: v/q/k for the first attention work; rest queued
        # as PE filler.  qt=0 only needs the nt=0 halves, so it can start
        # while the nt=1 operands are still streaming in.
        for tt in range(4):
            for _ in v_group(tt):
                pass
        for _ in qkv_group(qT_sb, wq_sb, 0, 0):
            pass
        for _ in qkv_group(kT_sb, wk_sb, 0, 0):
            pass
        for tt in range(4, NKT):
            for _ in v_group(tt):
                pass
        fillers.append(("q0n1", qkv_group(qT_sb, wq_sb, 0, 1)))
        fillers.append(("k0n1", qkv_group(kT_sb, wk_sb, 0, 1)))
        for m in range(1, NPAIR):
            fillers.append((f"k{m}n0", qkv_group(kT_sb, wk_sb, m, 0)))
            fillers.append((f"q{m}n0", qkv_group(qT_sb, wq_sb, m, 0)))
        for m in range(1, NPAIR):
            fillers.append((f"k{m}n1", qkv_group(kT_sb, wk_sb, m, 1)))
            fillers.append((f"q{m}n1", qkv_group(qT_sb, wq_sb, m, 1)))

        # ---- attention ----
        import os
        _V = os.environ.get("KV", "sbufmul")
        TAIL_SBUF = "sbufmul" in _V
        MASK2D = "mask2d" in _V

        def make_tail(m, outAB, q0):
            def tail():
                if TAIL_SBUF:
                    # sums rows -> bf16 (one op for both heads), K=1 broadcast
                    # matmuls, reciprocal into SBUF, multiply PSUM x SBUF
                    rr = sb_tmp.tile([65, 2, 512], BF16, name="rr")
                    nc.vector.tensor_copy(rr[64:65, :, :], outAB[64:65, :, :])
                    pump(2)
                    bcrs = []
                    for hh in range(2):
                        bc = ps_main.tile([64, 512], F32, name="ps")
                        nc.tensor.matmul(
                            bc[:], ones_c_sb[64:65, :], rr[64:65, hh, :],
                            start=True, stop=True, tile_position=(64, 0))
                        bcr = sb_tmp.tile([64, 512], F32, name="bcr")
                        nc.vector.reciprocal_approx_fast(out=bcr[:], in_=bc[:])
                        bcrs.append(bcr)
                    pump(1)
                    nc.vector.tensor_mul(
                        projT_sb[0:64, m, q0:q0 + 512],
                        outAB[0:64, 0, :], bcrs[0][:])
                    t2 = sb_tmp.tile([64, 512], BF16, name="t2")
                    nc.vector.tensor_mul(t2[:], outAB[0:64, 1, :], bcrs[1][:])
                else:
                    rr = sb_tmp.tile([65, 2, 512], F32, name="rr")
                    nc.vector.reciprocal_approx_fast(
                        out=rr[64:65, :, :], in_=outAB[64:65, :, :])
                    pump(2)
                    bcs = []
                    for hh in range(2):
                        bc = ps_main.tile([64, 512], F32, name="ps")
                        nc.tensor.matmul(
                            bc[:], ones_a_sb[64:65, :], rr[64:65, hh, :],
                            start=True, stop=True, tile_position=(64, 0))
                        bcs.append(bc)
                    pump(1)
                    nc.vector.tensor_mul(
                        projT_sb[0:64, m, q0:q0 + 512],
                        outAB[0:64, 0, :], bcs[0][:])
                    t2 = sb_tmp.tile([64, 512], BF16, name="t2")
                    nc.vector.tensor_mul(t2[:], outAB[0:64, 1, :], bcs[1][:])
                nc.sync.dma_start(
                    out=projT_sb[64:128, m, q0:q0 + 512], in_=t2[:])
            return tail

        for qt in (0, 1):
            q0 = 512 * qt
            pend_tail = None
            for m in range(NPAIR):
                flush({f"q{m}n{qt}", f"k{m}n0", f"k{m}n{qt}"})
                kts = list(range(4 * qt + 4))
                outAB = ps_out.tile([65, 2, 512], F32)
                pend = []
                for i in list(range(len(kts))) + [None]:
                    if i is not None:
                        kt = kts[i]
                        off = max(0, 128 * kt - q0)
                        w = 512 - off
                        qcols = slice(q0 + off, q0 + 512)
                        # score pair back-to-back so the row-tiled matmuls
                        # overlap on the PE
                        sAB = []
                        for hh, po in ((0, 0), (1, 64)):
                            s = ps_main.tile(
                                [128, 512], F32, name="ps")[:, :w]
                            nc.tensor.matmul(
                                s,
                                kT_sb[po:po + 64, m, ts(kt, 128)],
                                qT_sb[po:po + 64, m, qcols],
                                start=True, stop=True,
                                tile_position=(po, 0))
                            sAB.append(s)
                        eAB = sb_exp.tile([128, 2, 512], BF16, name="et")
                        for hh in range(2):
                            nc.scalar.activation(
                                eAB[:, hh, :w], sAB[hh], AF.Exp, scale=0.125)
                        if kt >= 4 * qt:  # diagonal: zero upper triangle
                            if MASK2D:
                                for hh in range(2):
                                    nc.gpsimd.tensor_mul(
                                        eAB[:, hh, 0:128], eAB[:, hh, 0:128],
                                        tri_sb[:, 0, :])
                            else:
                                nc.gpsimd.tensor_mul(
                                    eAB[:, :, 0:128], eAB[:, :, 0:128],
                                    tri_sb[:, :, :])
                        pend.append((eAB, i, off))
                    pump(1)
                    if len(pend) > 1 or (i is None and pend):
                        (eAB_p, pi, poff) = pend.pop(0)
                        pw = 512 - poff
                        for hh in range(2):
                            nc.tensor.matmul(
                                outAB[0:65, hh, poff:512],
                                v_sb[:, kts[pi], 2 * m + hh, :],
                                eAB_p[:, hh, 0:pw],
                                start=(pi == 0), stop=(pi == len(kts) - 1))
                    # previous pair's normalize, interleaved into this loop
                    if pend_tail is not None and (i == 1 or i is None):
                        pend_tail()
                        pend_tail = None
                pend_tail = make_tail(m, outAB, q0)
            pend_tail()
            pend_tail = None
            # queue this q-half's projection as PE filler for the next phase
            for tt2 in range(4):
                for n2 in range(2):
                    fillers.append((f"p{qt}", proj_group(
                        q0, tt2, n2, drain=(qt == 1))))
        # drain remaining projection work
        while fillers:
            pump(len(fillers) * 8)

    nc.compile()
    return nc


def _program():
    if "nc" not in _CACHE:
        _CACHE["nc"] = _build()
    return _CACHE["nc"]


def _bf(a):
    return np.ascontiguousarray(a).astype(ml_dtypes.bfloat16)


def _in_maps(x, Wq, Wk, Wv, Wproj):
    tri1 = np.triu(np.ones((128, 128), dtype=np.float32))  # tri[p,f]=1, f>=p
    tri = np.ascontiguousarray(np.concatenate([tri1, tri1], axis=1)
                               ).astype(ml_dtypes.bfloat16)
    ones_a = np.ones((1, 64), dtype=np.float32)
    ones_c = np.ones((1, 64), dtype=ml_dtypes.bfloat16)
    ones_b = np.ones((128, 8), dtype=ml_dtypes.bfloat16)
    maps = []
    for c in range(N_CORES):
        b, g = c // 2, c % 2
        sl = slice(CL * g, CL * (g + 1))
        maps.append({
            "xt": _bf(x[b].T),
            "wq": _bf(Wq[:, sl]),
            "wk": _bf(Wk[:, sl]),
            "wv": _bf(Wv[:, sl]),
            "wp": _bf(Wproj[sl, :]),
            "tri": tri,
            "ones_a": ones_a,
            "ones_c": ones_c,
            "ones_b": ones_b,
        })
    return maps


def run(x, Wq, Wk, Wv, Wproj, trace=False, **kwargs):
    nc = _program()
    maps = _in_maps(np.asarray(x, dtype=np.float32),
                    np.asarray(Wq, dtype=np.float32),
                    np.asarray(Wk, dtype=np.float32),
                    np.asarray(Wv, dtype=np.float32),
                    np.asarray(Wproj, dtype=np.float32))
    res = run_bass_kernel_spmd(nc, maps, core_ids=list(range(N_CORES)),
                               trace=trace, **kwargs)
    B = 4
    out = np.empty((B, T, C), dtype=np.float32)
    for b in range(B):
        out[b] = (res.results[2 * b]["y"].astype(np.float32)
                  + res.results[2 * b + 1]["y"].astype(np.float32))
    return out, res


def kernel(x, Wq, Wk, Wv, Wproj):
    out, _ = run(x, Wq, Wk, Wv, Wproj)
    return out
